# revision 3
# baseline (speedup 1.0000x reference)
"""Bilinear interpolation (affine scale+translate sampling) on 8 Trainium2 NeuronCores.

Contract: kernel(X, scale, translate) -> np.ndarray [16, 512, 512, 16] float32,
matching reference.py's bilinear sampler.

Math: the affine is [[s,0,tx],[0,s,ty]] -> x coords depend only on output col j,
y coords only on output row i. Bilinear sampling therefore factorizes into two
1-D resampling passes, each a banded matrix multiply:

  out[i,j,c] = sum_h BT[h,i] * ( sum_w X[h,w,c] * AT[w,j] )

with BT/AT having <=2 nonzeros per column (the two interpolation taps).
Both passes run on the TensorEngine:
  pass 1 (V^T): for each channel c, V^T[w, i] = sum_h X[h,w,c] * BT[h,i]
    (lhsT = X tile [h,w] is the stationary operand -> output lands w-on-partitions)
  pass 2 (H):   out[i, j]_c = sum_w V^T[w, i] * AT[w, j]
    (lhsT = V^T tile [w,i] stationary -> output lands i-on-partitions, row-major)

Each of the 16 batches has its own geometry (valid output rect, input rect,
tile counts) baked statically into its own section of ONE SPMD program; each of
the 8 cores selects its (<=2) batch sections via a binary If-tree on
partition_id. Out-of-bounds output regions are exactly zero (weights cancel)
and are never touched (outputs are zero-initialized).
"""
import hashlib
import os
import sys
import numpy as np

_EXTRA_PATHS = [
    "/root/.axon_site",
    "/root/.axon_site/_ro/trn_rl_repo",
    "/root/.axon_site/_ro/pypackages",
    "/opt/trn_rl_repo",
]
for _p in _EXTRA_PATHS:
    if _p not in sys.path and os.path.isdir(_p):
        sys.path.append(_p)

import concourse.bass as bass
import concourse.bacc as bacc
import concourse.mybir as mybir
import concourse.tile as tile
from concourse.bass_utils import run_bass_kernel_spmd

B, H, W, C = 16, 512, 512, 16
OH, OW = 512, 512
NCORES = 8
P = 128
MAXT = 4          # max 128-row/col tiles per axis
H_DTYPE = os.environ.get("BILIN_H_DTYPE", "fp32")   # "fp32" | "fp32r"
NEFF_CACHE_DIR = os.environ.get(
    "BILIN_NEFF_CACHE", os.path.expanduser("~/.cache/bilin_neff")
)

_f32 = np.float32


# ----------------------------------------------------------------------------
# host-side planning (exact fp32 mirror of the reference coordinate math)
# ----------------------------------------------------------------------------

def _axis_plan(s, t, size, n):
    """Coordinates along one output axis. Mirrors reference.py in fp32."""
    lin = np.linspace(-1.0, 1.0, n).astype(np.float32)
    sg = (_f32(s) * lin + _f32(t)).astype(np.float32)
    v = (_f32(0.5) * (sg + _f32(1.0)) * _f32(size)).astype(np.float32)
    i0 = v.astype(np.int32)
    i1 = i0 + 1
    i0c = np.clip(i0, 0, size - 1)
    i1c = np.clip(i1, 0, size - 1)
    f0 = i0c.astype(np.float32)
    f1 = i1c.astype(np.float32)
    w0 = (f1 - v).astype(np.float32)
    w1 = (v - f0).astype(np.float32)
    valid = i1c == i0c + 1
    idx = np.nonzero(valid)[0]
    if len(idx) == 0:
        return None
    lo, hi = int(idx[0]), int(idx[-1]) + 1
    assert valid[lo:hi].all(), "valid output range is not contiguous"
    return dict(i0=i0c, i1=i1c, w0=w0, w1=w1, lo=lo, hi=hi,
                mlo=int(i0c[lo:hi].min()), mhi=int(i1c[lo:hi].max()))


def _plan_batch(s, tx, ty):
    """Full plan for one batch, or None if the output is entirely zero."""
    px = _axis_plan(s, tx, W, OW)
    py = _axis_plan(s, ty, H, OH)
    if px is None or py is None:
        return None
    jl, jr, wlo, whi = px["lo"], px["hi"], px["mlo"], px["mhi"]
    il, ir, hlo, hhi = py["lo"], py["hi"], py["mlo"], py["mhi"]
    nj, nw = jr - jl, whi - wlo + 1
    ni, nh = ir - il, hhi - hlo + 1
    Th = -(-nh // P)
    Wb = -(-nw // P)

    # vertical weights: BT[t, r, k] with r = h - hlo within tile t, k = i - il
    rows0 = py["i0"][il:ir].astype(np.int64) - hlo          # monotone
    rows1 = rows0 + 1
    ar = np.arange(ni)
    BT = np.zeros((MAXT, P, 512), dtype=np.float32)
    flat = np.zeros((MAXT * P, 512), dtype=np.float32)
    flat[rows0, ar] += py["w0"][il:ir]
    flat[rows1, ar] += py["w1"][il:ir]
    BT[:, :, :] = flat.reshape(MAXT, P, 512)

    # horizontal weights: AT[t, r, j] with r = w - wlo within tile t, j = j - jl
    cols0 = px["i0"][jl:jr].astype(np.int64) - wlo
    cols1 = cols0 + 1
    aj = np.arange(nj)
    AT = np.zeros((MAXT, P, 512), dtype=np.float32)
    flat = np.zeros((MAXT * P, 512), dtype=np.float32)
    flat[cols0, aj] += px["w0"][jl:jr]
    flat[cols1, aj] += px["w1"][jl:jr]
    AT[:, :, :] = flat.reshape(MAXT, P, 512)

    # sub-ranges of i touched by vertical tile t (for t >= 1 partial matmuls)
    vranges = []
    for t in range(Th):
        kA = int(np.searchsorted(rows1, t * P, side="left"))
        kB = int(np.searchsorted(rows0, (t + 1) * P, side="left"))
        vranges.append((kA, kB))
    hranges = []
    for t in range(Wb):
        jA = int(np.searchsorted(cols1, t * P, side="left"))
        jB = int(np.searchsorted(cols0, (t + 1) * P, side="left"))
        hranges.append((jA, jB))

    # split the valid-i range to bound SBUF (V^T intermediate + weights)
    n_isplit = 2 if ni > 256 else 1

    # rough fp32 PE cost (cycles) for bin-packing
    vcyc = Wb * C * (ni + sum(b - a for a, b in vranges[1:])) * 4
    hcyc = (-(-ni // P)) * C * (nj + sum(b - a for a, b in hranges[1:])) * 4
    cost = (vcyc + hcyc) / 2400.0 + (nh * nw + ni * nj) * 64 / 405e3  # us

    return dict(jl=jl, jr=jr, wlo=wlo, whi=whi, il=il, ir=ir, hlo=hlo, hhi=hhi,
                nj=nj, nw=nw, ni=ni, nh=nh, Th=Th, Wb=Wb, BT=BT, AT=AT,
                vranges=vranges, hranges=hranges, n_isplit=n_isplit, cost=cost)


def _binpack(plans):
    """Assign batches to 8 cores (<=2 each), balancing estimated cost.
    Returns core_batches: list of 8 lists of batch indices."""
    active = [(p["cost"], b) for b, p in enumerate(plans) if p is not None]
    active.sort(reverse=True)
    loads = [0.0] * NCORES
    slots = [[] for _ in range(NCORES)]
    for cost, b in active:
        k = min((k for k in range(NCORES) if len(slots[k]) < 2),
                key=lambda k: loads[k])
        slots[k].append(b)
        loads[k] += cost
    # zero batches: not assigned anywhere (no device work)
    return slots


# ----------------------------------------------------------------------------
# device program
# ----------------------------------------------------------------------------

def _emit_batch(nc, tc, pools, ios, slot, pl):
    """Emit the device program for one batch (static geometry from pl)."""
    sbuf, psum = pools
    XR_in, BT_in, AT_in, OUT = ios
    f32 = mybir.dt.float32
    hdt = mybir.dt.float32r if H_DTYPE == "fp32r" else f32
    Th, Wb, ni, nj = pl["Th"], pl["Wb"], pl["ni"], pl["nj"]
    nwp16 = Wb * P * 16   # w-block-padded row width (host zero-pads)

    # stage input rect tiles (zero-padded by host)
    xr = []
    for t in range(Th):
        xt = sbuf.tile([P, nwp16], f32, tag=f"xr{t}", name=f"xr{t}_{slot}")
        nc.sync.dma_start(xt[:], XR_in[slot, t, :, 0:nwp16])
        xr.append(xt)
    bts = []
    for t in range(Th):
        bt = sbuf.tile([P, 512], f32, tag=f"bt{t}", name=f"bt{t}_{slot}")
        nc.sync.dma_start(bt[:], BT_in[slot, t, :, :])
        bts.append(bt)
    ats = []
    for t in range(Wb):
        at_f = sbuf.tile([P, 512], f32, tag=f"atf{t}", name=f"atf{t}_{slot}")
        nc.sync.dma_start(at_f[:], AT_in[slot, t, :, :])
        if hdt != f32:
            at_r = sbuf.tile([P, 512], hdt, tag=f"atr{t}", name=f"atr{t}_{slot}")
            nc.vector.tensor_copy(at_r[:], at_f[:])
            ats.append(at_r)
        else:
            ats.append(at_f)

    n_split = pl["n_isplit"]
    bounds = [(ni * q) // n_split for q in range(n_split + 1)]
    cp = [0]  # copyout engine round-robin

    def copyout(dst_ap, src_ap):
        if cp[0] % 2 == 0:
            nc.vector.tensor_copy(dst_ap, src_ap)
        else:
            nc.scalar.copy(dst_ap, src_ap)
        cp[0] += 1

    for q in range(n_split):
        iA, iB = bounds[q], bounds[q + 1]
        nis = iB - iA
        # ---- pass 1: V^T[w, i]_c for i in [iA, iB) ----
        vts = []
        for wb in range(Wb):
            vt = sbuf.tile([P, 16 * 256], hdt, tag=f"vt{wb}",
                           name=f"vt{wb}_{slot}_{q}")
            vts.append(vt)
        for c in range(C):
            for wb in range(Wb):
                pv = psum.tile([P, 512], f32, tag="psv",
                               name=f"psv_{slot}_{q}_{c}_{wb}")
                active = [t for t in range(1, Th)
                          if max(pl["vranges"][t][0], iA) < min(pl["vranges"][t][1], iB)]
                last_t = active[-1] if active else 0
                for t in [0] + active:
                    if t == 0:
                        kA, kB = iA, iB
                    else:
                        kA, kB = pl["vranges"][t]
                        kA, kB = max(kA, iA), min(kB, iB)
                    w0 = wb * P
                    nc.tensor.matmul(
                        pv[:, kA - iA:kB - iA],
                        lhsT=xr[t][:, w0 * 16 + c: (w0 + P - 1) * 16 + c + 1: 16],
                        rhs=bts[t][:, kA:kB],
                        start=(t == 0), stop=(t == last_t),
                    )
                copyout(vts[wb][:, c * nis:(c + 1) * nis], pv[:, 0:nis])

        # ---- pass 2: out[i, j]_c for i-blocks in [iA, iB) ----
        nib = -(-nis // P)
        for ib in range(nib):
            r0 = ib * P
            ilen = min(P, nis - r0)
            ot = sbuf.tile([P, 8192], f32, tag="out", name=f"out_{slot}_{q}_{ib}")
            for c in range(C):
                ph = psum.tile([P, 512], f32, tag="psh",
                               name=f"psh_{slot}_{q}_{ib}_{c}")
                active = [t for t in range(1, Wb)
                          if pl["hranges"][t][0] < pl["hranges"][t][1]]
                last_t = active[-1] if active else 0
                for t in [0] + active:
                    jA, jB = (0, nj) if t == 0 else pl["hranges"][t]
                    nc.tensor.matmul(
                        ph[0:ilen, jA:jB],
                        lhsT=vts[t][:, c * nis + r0: c * nis + r0 + ilen],
                        rhs=ats[t][:, jA:jB],
                        start=(t == 0), stop=(t == last_t),
                    )
                copyout(ot[0:ilen, c: c + 16 * (nj - 1) + 1: 16],
                        ph[0:ilen, 0:nj])
            nc.sync.dma_start(
                OUT[slot, pl["il"] + iA + r0: pl["il"] + iA + r0 + ilen,
                    pl["jl"]:pl["jr"], :],
                ot[0:ilen, 0:nj * 16],
            )


def _build_program(plans, core_batches):
    nc = bacc.Bacc("TRN2", target_bir_lowering=False, debug=False)
    f32 = mybir.dt.float32
    XR_in = nc.dram_tensor("xr_in", [2, MAXT, P, 8192], f32, kind="ExternalInput").ap()
    BT_in = nc.dram_tensor("bt_in", [2, MAXT, P, 512], f32, kind="ExternalInput").ap()
    AT_in = nc.dram_tensor("at_in", [2, MAXT, P, 512], f32, kind="ExternalInput").ap()
    OUT = nc.dram_tensor("out", [2, OH, OW, C], f32, kind="ExternalOutput").ap()

    with tile.TileContext(nc) as tc:
        with (
            tc.tile_pool(name="sbuf", bufs=1) as sbuf,
            tc.tile_pool(name="psum", bufs=2, space="PSUM") as psum,
        ):
            ios = (XR_in, BT_in, AT_in, OUT)
            pools = (sbuf, psum)
            pid = nc.partition_id()

            def section(k):
                for slot, b in enumerate(core_batches[k]):
                    _emit_batch(nc, tc, pools, ios, slot, plans[b])

            def tree(lo, hi):
                if hi - lo == 1:
                    if core_batches[lo]:
                        section(lo)
                    return
                mid = (lo + hi) // 2
                with tc.If(pid < mid) as cmp:
                    tree(lo, mid)
                with cmp.Else():
                    tree(mid, hi)

            tree(0, NCORES)
    nc.compile()
    return nc


# ----------------------------------------------------------------------------
# NEFF disk cache (patches concourse's compile path; affects this process only)
# ----------------------------------------------------------------------------

def _install_neff_cache():
    import concourse.bass_utils as bu
    import concourse.bass2jax as b2j
    if getattr(bu, "_bilin_cache_installed", False):
        return
    orig = bu.compile_bir_kernel

    def cached(bir_json, tmpdir, neff_name="file.neff"):
        try:
            os.makedirs(NEFF_CACHE_DIR, exist_ok=True)
            key = hashlib.sha256(bir_json).hexdigest()[:32]
            path = os.path.join(NEFF_CACHE_DIR, key + ".neff")
            if os.path.exists(path):
                dst = os.path.join(tmpdir, neff_name)
                import shutil
                shutil.copy(path, dst)
                return dst
            out = orig(bir_json, tmpdir, neff_name)
            import shutil
            shutil.copy(out, path)
            return out
        except Exception:
            return orig(bir_json, tmpdir, neff_name)

    bu.compile_bir_kernel = cached
    b2j.compile_bir_kernel = cached
    bu._bilin_cache_installed = True


# ----------------------------------------------------------------------------
# entry point
# ----------------------------------------------------------------------------

_prog_cache = {}


def kernel(X, scale, translate):
    X = np.ascontiguousarray(np.asarray(X, dtype=np.float32))
    scale = np.asarray(scale, dtype=np.float32)
    translate = np.asarray(translate, dtype=np.float32)
    assert X.shape == (B, H, W, C)

    plans = [
        _plan_batch(float(scale[b, 0]), float(translate[b, 0]), float(translate[b, 1]))
        for b in range(B)
    ]
    core_batches = _binpack(plans)

    key = (scale.tobytes(), translate.tobytes(), H_DTYPE)
    if key in _prog_cache:
        nc, core_batches = _prog_cache[key]
    else:
        _install_neff_cache()
        nc = _build_program(plans, core_batches)
        _prog_cache[key] = (nc, core_batches)

    # per-core inputs
    in_maps = []
    for k in range(NCORES):
        XRk = np.zeros((2, MAXT, P, 8192), dtype=np.float32)
        BTk = np.zeros((2, MAXT, P, 512), dtype=np.float32)
        ATk = np.zeros((2, MAXT, P, 512), dtype=np.float32)
        for slot, b in enumerate(core_batches[k]):
            pl = plans[b]
            nw16 = pl["nw"] * 16
            for t in range(pl["Th"]):
                r0 = pl["hlo"] + t * P
                r1 = min(r0 + P, pl["hhi"] + 1)
                rect = X[b, r0:r1, pl["wlo"]:pl["whi"] + 1, :].reshape(r1 - r0, nw16)
                XRk[slot, t, 0:r1 - r0, 0:nw16] = rect
            BTk[slot] = pl["BT"]
            ATk[slot] = pl["AT"]
        in_maps.append({"xr_in": XRk, "bt_in": BTk, "at_in": ATk})

    res = run_bass_kernel_spmd(nc, in_maps, core_ids=list(range(NCORES)))

    out = np.zeros((B, OH, OW, C), dtype=np.float32)
    for k in range(NCORES):
        for slot, b in enumerate(core_batches[k]):
            out[b] = res.results[k]["out"][slot]
    return out


# revision 4
# speedup vs baseline: 206.4742x; 206.4742x over previous
"""Bilinear interpolation (affine scale+translate sampling) on 8 Trainium2 NeuronCores.

Contract: kernel(X, scale, translate) -> np.ndarray [16, 512, 512, 16] float32,
matching reference.py's bilinear sampler.

Math: the affine is [[s,0,tx],[0,s,ty]] -> x coords depend only on output col j,
y coords only on output row i. Bilinear sampling therefore factorizes into two
1-D resampling passes, each a banded matrix multiply:

  out[i,j,c] = sum_h BT[h,i] * ( sum_w X[h,w,c] * AT[w,j] )

with BT/AT having <=2 nonzeros per column (the two interpolation taps).
Both passes run on the TensorEngine:
  pass 1 (V^T): for each channel c, V^T[w, i] = sum_h X[h,w,c] * BT[h,i]
    (lhsT = X tile [h,w] is the stationary operand -> output lands w-on-partitions)
  pass 2 (H):   out[i, j]_c = sum_w V^T[w, i] * AT[w, j]
    (lhsT = V^T tile [w,i] stationary -> output lands i-on-partitions, row-major)

Each of the 16 batches has its own geometry (valid output rect, input rect,
tile counts) baked statically into its own section of ONE SPMD program; each of
the 8 cores selects its (<=2) batch sections via a binary If-tree on
partition_id. Out-of-bounds output regions are exactly zero (weights cancel)
and are never touched (outputs are zero-initialized).
"""
import hashlib
import os
import sys
import numpy as np

_EXTRA_PATHS = [
    "/root/.axon_site",
    "/root/.axon_site/_ro/trn_rl_repo",
    "/root/.axon_site/_ro/pypackages",
    "/opt/trn_rl_repo",
]
for _p in _EXTRA_PATHS:
    if _p not in sys.path and os.path.isdir(_p):
        sys.path.append(_p)

import concourse.bass as bass
import concourse.bacc as bacc
import concourse.mybir as mybir
import concourse.tile as tile
from concourse.bass_utils import run_bass_kernel_spmd

B, H, W, C = 16, 512, 512, 16
OH, OW = 512, 512
NCORES = 8
P = 128
MAXT = 4          # max 128-row/col tiles per axis
H_DTYPE = os.environ.get("BILIN_H_DTYPE", "fp32")   # "fp32" | "fp32r"
REPEAT = int(os.environ.get("BILIN_REPEAT", "1"))    # replicate device work (timing)
NEFF_CACHE_DIR = os.environ.get(
    "BILIN_NEFF_CACHE", os.path.expanduser("~/.cache/bilin_neff")
)

_f32 = np.float32


# ----------------------------------------------------------------------------
# host-side planning (exact fp32 mirror of the reference coordinate math)
# ----------------------------------------------------------------------------

def _axis_plan(s, t, size, n):
    """Coordinates along one output axis. Mirrors reference.py in fp32."""
    lin = np.linspace(-1.0, 1.0, n).astype(np.float32)
    sg = (_f32(s) * lin + _f32(t)).astype(np.float32)
    v = (_f32(0.5) * (sg + _f32(1.0)) * _f32(size)).astype(np.float32)
    i0 = v.astype(np.int32)
    i1 = i0 + 1
    i0c = np.clip(i0, 0, size - 1)
    i1c = np.clip(i1, 0, size - 1)
    f0 = i0c.astype(np.float32)
    f1 = i1c.astype(np.float32)
    w0 = (f1 - v).astype(np.float32)
    w1 = (v - f0).astype(np.float32)
    valid = i1c == i0c + 1
    idx = np.nonzero(valid)[0]
    if len(idx) == 0:
        return None
    lo, hi = int(idx[0]), int(idx[-1]) + 1
    assert valid[lo:hi].all(), "valid output range is not contiguous"
    return dict(i0=i0c, i1=i1c, w0=w0, w1=w1, lo=lo, hi=hi,
                mlo=int(i0c[lo:hi].min()), mhi=int(i1c[lo:hi].max()))


def _plan_batch(s, tx, ty):
    """Full plan for one batch, or None if the output is entirely zero."""
    px = _axis_plan(s, tx, W, OW)
    py = _axis_plan(s, ty, H, OH)
    if px is None or py is None:
        return None
    jl, jr, wlo, whi = px["lo"], px["hi"], px["mlo"], px["mhi"]
    il, ir, hlo, hhi = py["lo"], py["hi"], py["mlo"], py["mhi"]
    nj, nw = jr - jl, whi - wlo + 1
    ni, nh = ir - il, hhi - hlo + 1
    Th = -(-nh // P)
    Wb = -(-nw // P)

    # vertical weights: BT[t, r, k] with r = h - hlo within tile t, k = i - il
    rows0 = py["i0"][il:ir].astype(np.int64) - hlo          # monotone
    rows1 = rows0 + 1
    ar = np.arange(ni)
    BT = np.zeros((MAXT, P, 512), dtype=np.float32)
    flat = np.zeros((MAXT * P, 512), dtype=np.float32)
    flat[rows0, ar] += py["w0"][il:ir]
    flat[rows1, ar] += py["w1"][il:ir]
    BT[:, :, :] = flat.reshape(MAXT, P, 512)

    # horizontal weights: AT[t, r, j] with r = w - wlo within tile t, j = j - jl
    cols0 = px["i0"][jl:jr].astype(np.int64) - wlo
    cols1 = cols0 + 1
    aj = np.arange(nj)
    AT = np.zeros((MAXT, P, 512), dtype=np.float32)
    flat = np.zeros((MAXT * P, 512), dtype=np.float32)
    flat[cols0, aj] += px["w0"][jl:jr]
    flat[cols1, aj] += px["w1"][jl:jr]
    AT[:, :, :] = flat.reshape(MAXT, P, 512)

    # sub-ranges of i touched by vertical tile t (for t >= 1 partial matmuls)
    vranges = []
    for t in range(Th):
        kA = int(np.searchsorted(rows1, t * P, side="left"))
        kB = int(np.searchsorted(rows0, (t + 1) * P, side="left"))
        vranges.append((kA, kB))
    hranges = []
    for t in range(Wb):
        jA = int(np.searchsorted(cols1, t * P, side="left"))
        jB = int(np.searchsorted(cols0, (t + 1) * P, side="left"))
        hranges.append((jA, jB))

    # split the valid-i range to bound SBUF (V^T intermediate + weights)
    n_isplit = 2 if ni > 256 else 1

    # rough fp32 PE cost (cycles) for bin-packing
    vcyc = Wb * C * (ni + sum(b - a for a, b in vranges[1:])) * 4
    hcyc = (-(-ni // P)) * C * (nj + sum(b - a for a, b in hranges[1:])) * 4
    cost = (vcyc + hcyc) / 2400.0 + (nh * nw + ni * nj) * 64 / 405e3  # us

    return dict(jl=jl, jr=jr, wlo=wlo, whi=whi, il=il, ir=ir, hlo=hlo, hhi=hhi,
                nj=nj, nw=nw, ni=ni, nh=nh, Th=Th, Wb=Wb, BT=BT, AT=AT,
                vranges=vranges, hranges=hranges, n_isplit=n_isplit, cost=cost)


def _binpack(plans):
    """Assign batches to 8 cores (<=2 each), balancing estimated cost.
    Returns core_batches: list of 8 lists of batch indices."""
    active = [(p["cost"], b) for b, p in enumerate(plans) if p is not None]
    active.sort(reverse=True)
    loads = [0.0] * NCORES
    slots = [[] for _ in range(NCORES)]
    for cost, b in active:
        k = min((k for k in range(NCORES) if len(slots[k]) < 2),
                key=lambda k: loads[k])
        slots[k].append(b)
        loads[k] += cost
    # zero batches: not assigned anywhere (no device work)
    return slots


# ----------------------------------------------------------------------------
# device program
# ----------------------------------------------------------------------------

def _emit_batch(nc, tc, pools, ios, slot, pl):
    """Emit the device program for one batch (static geometry from pl)."""
    sbuf, psum = pools
    XR_in, BT_in, AT_in, OUT = ios
    f32 = mybir.dt.float32
    hdt = mybir.dt.float32r if H_DTYPE == "fp32r" else f32
    Th, Wb, ni, nj = pl["Th"], pl["Wb"], pl["ni"], pl["nj"]
    nwp16 = Wb * P * 16   # w-block-padded row width (host zero-pads)

    # stage input rect tiles (zero-padded by host)
    xr = []
    for t in range(Th):
        xt = sbuf.tile([P, nwp16], f32, tag=f"xr{t}", name=f"xr{t}_{slot}")
        nc.sync.dma_start(xt[:], XR_in[slot, t, :, 0:nwp16])
        xr.append(xt)
    bts = []
    for t in range(Th):
        bt = sbuf.tile([P, 512], f32, tag=f"bt{t}", name=f"bt{t}_{slot}")
        nc.sync.dma_start(bt[:], BT_in[slot, t, :, :])
        bts.append(bt)
    ats = []
    for t in range(Wb):
        at_f = sbuf.tile([P, 512], f32, tag=f"atf{t}", name=f"atf{t}_{slot}")
        nc.sync.dma_start(at_f[:], AT_in[slot, t, :, :])
        if hdt != f32:
            at_r = sbuf.tile([P, 512], hdt, tag=f"atr{t}", name=f"atr{t}_{slot}")
            nc.vector.tensor_copy(at_r[:], at_f[:])
            ats.append(at_r)
        else:
            ats.append(at_f)

    n_split = pl["n_isplit"]
    bounds = [(ni * q) // n_split for q in range(n_split + 1)]
    cp = [0]  # copyout engine round-robin

    def copyout(dst_ap, src_ap):
        if cp[0] % 2 == 0:
            nc.vector.tensor_copy(dst_ap, src_ap)
        else:
            nc.scalar.copy(dst_ap, src_ap)
        cp[0] += 1

    for q in range(n_split):
        iA, iB = bounds[q], bounds[q + 1]
        nis = iB - iA
        # ---- pass 1: V^T[w, i]_c for i in [iA, iB) ----
        vts = []
        for wb in range(Wb):
            vt = sbuf.tile([P, 16 * 256], hdt, tag=f"vt{wb}",
                           name=f"vt{wb}_{slot}_{q}")
            vts.append(vt)
        for c in range(C):
            for wb in range(Wb):
                pv = psum.tile([P, 512], f32, tag="psv",
                               name=f"psv_{slot}_{q}_{c}_{wb}")
                active = [t for t in range(1, Th)
                          if max(pl["vranges"][t][0], iA) < min(pl["vranges"][t][1], iB)]
                last_t = active[-1] if active else 0
                for t in [0] + active:
                    if t == 0:
                        kA, kB = iA, iB
                    else:
                        kA, kB = pl["vranges"][t]
                        kA, kB = max(kA, iA), min(kB, iB)
                    w0 = wb * P
                    nc.tensor.matmul(
                        pv[:, kA - iA:kB - iA],
                        lhsT=xr[t][:, w0 * 16 + c: (w0 + P - 1) * 16 + c + 1: 16],
                        rhs=bts[t][:, kA:kB],
                        start=(t == 0), stop=(t == last_t),
                    )
                copyout(vts[wb][:, c * nis:(c + 1) * nis], pv[:, 0:nis])

        # ---- pass 2: out[i, j]_c for i-blocks in [iA, iB) ----
        nib = -(-nis // P)
        for ib in range(nib):
            r0 = ib * P
            ilen = min(P, nis - r0)
            ot = sbuf.tile([P, 8192], f32, tag="out", name=f"out_{slot}_{q}_{ib}")
            for c in range(C):
                ph = psum.tile([P, 512], f32, tag="psh",
                               name=f"psh_{slot}_{q}_{ib}_{c}")
                active = [t for t in range(1, Wb)
                          if pl["hranges"][t][0] < pl["hranges"][t][1]]
                last_t = active[-1] if active else 0
                for t in [0] + active:
                    jA, jB = (0, nj) if t == 0 else pl["hranges"][t]
                    nc.tensor.matmul(
                        ph[0:ilen, jA:jB],
                        lhsT=vts[t][:, c * nis + r0: c * nis + r0 + ilen],
                        rhs=ats[t][:, jA:jB],
                        start=(t == 0), stop=(t == last_t),
                    )
                copyout(ot[0:ilen, c: c + 16 * (nj - 1) + 1: 16],
                        ph[0:ilen, 0:nj])
            nc.sync.dma_start(
                OUT[slot, pl["il"] + iA + r0: pl["il"] + iA + r0 + ilen,
                    pl["jl"]:pl["jr"], :],
                ot[0:ilen, 0:nj * 16],
            )


def _build_program(plans, core_batches):
    nc = bacc.Bacc("TRN2", target_bir_lowering=False, debug=False)
    f32 = mybir.dt.float32
    XR_in = nc.dram_tensor("xr_in", [2, MAXT, P, 8192], f32, kind="ExternalInput").ap()
    BT_in = nc.dram_tensor("bt_in", [2, MAXT, P, 512], f32, kind="ExternalInput").ap()
    AT_in = nc.dram_tensor("at_in", [2, MAXT, P, 512], f32, kind="ExternalInput").ap()
    OUT = nc.dram_tensor("out", [2, OH, OW, C], f32, kind="ExternalOutput").ap()

    with tile.TileContext(nc) as tc:
        with (
            tc.tile_pool(name="sbuf", bufs=1) as sbuf,
            tc.tile_pool(name="psum", bufs=2, space="PSUM") as psum,
        ):
            ios = (XR_in, BT_in, AT_in, OUT)
            pools = (sbuf, psum)
            pid = nc.partition_id()

            def section(k):
                for _r in range(REPEAT):
                    for slot, b in enumerate(core_batches[k]):
                        _emit_batch(nc, tc, pools, ios, slot, plans[b])

            def tree(lo, hi):
                if hi - lo == 1:
                    if core_batches[lo]:
                        section(lo)
                    return
                mid = (lo + hi) // 2
                with tc.If(pid < mid) as cmp:
                    tree(lo, mid)
                with cmp.Else():
                    tree(mid, hi)

            tree(0, NCORES)
    nc.compile()
    return nc


# ----------------------------------------------------------------------------
# NEFF disk cache (patches concourse's compile path; affects this process only)
# ----------------------------------------------------------------------------

def _install_neff_cache():
    import concourse.bass_utils as bu
    import concourse.bass2jax as b2j
    if getattr(bu, "_bilin_cache_installed", False):
        return
    orig = bu.compile_bir_kernel

    def cached(bir_json, tmpdir, neff_name="file.neff"):
        try:
            os.makedirs(NEFF_CACHE_DIR, exist_ok=True)
            key = hashlib.sha256(bir_json).hexdigest()[:32]
            path = os.path.join(NEFF_CACHE_DIR, key + ".neff")
            if os.path.exists(path):
                dst = os.path.join(tmpdir, neff_name)
                import shutil
                shutil.copy(path, dst)
                return dst
            out = orig(bir_json, tmpdir, neff_name)
            import shutil
            shutil.copy(out, path)
            return out
        except Exception:
            return orig(bir_json, tmpdir, neff_name)

    bu.compile_bir_kernel = cached
    b2j.compile_bir_kernel = cached
    bu._bilin_cache_installed = True


# ----------------------------------------------------------------------------
# entry point
# ----------------------------------------------------------------------------

_prog_cache = {}


def kernel(X, scale, translate):
    X = np.ascontiguousarray(np.asarray(X, dtype=np.float32))
    scale = np.asarray(scale, dtype=np.float32)
    translate = np.asarray(translate, dtype=np.float32)
    assert X.shape == (B, H, W, C)

    plans = [
        _plan_batch(float(scale[b, 0]), float(translate[b, 0]), float(translate[b, 1]))
        for b in range(B)
    ]
    core_batches = _binpack(plans)

    key = (scale.tobytes(), translate.tobytes(), H_DTYPE, REPEAT)
    if key in _prog_cache:
        nc, core_batches = _prog_cache[key]
    else:
        _install_neff_cache()
        nc = _build_program(plans, core_batches)
        _prog_cache[key] = (nc, core_batches)

    # per-core inputs
    in_maps = []
    for k in range(NCORES):
        XRk = np.zeros((2, MAXT, P, 8192), dtype=np.float32)
        BTk = np.zeros((2, MAXT, P, 512), dtype=np.float32)
        ATk = np.zeros((2, MAXT, P, 512), dtype=np.float32)
        for slot, b in enumerate(core_batches[k]):
            pl = plans[b]
            nw16 = pl["nw"] * 16
            for t in range(pl["Th"]):
                r0 = pl["hlo"] + t * P
                r1 = min(r0 + P, pl["hhi"] + 1)
                rect = X[b, r0:r1, pl["wlo"]:pl["whi"] + 1, :].reshape(r1 - r0, nw16)
                XRk[slot, t, 0:r1 - r0, 0:nw16] = rect
            BTk[slot] = pl["BT"]
            ATk[slot] = pl["AT"]
        in_maps.append({"xr_in": XRk, "bt_in": BTk, "at_in": ATk})

    res = run_bass_kernel_spmd(nc, in_maps, core_ids=list(range(NCORES)))

    out = np.zeros((B, OH, OW, C), dtype=np.float32)
    for k in range(NCORES):
        for slot, b in enumerate(core_batches[k]):
            out[b] = res.results[k]["out"][slot]
    return out
